# revision 9
# baseline (speedup 1.0000x reference)
"""Trainium2 Bass kernel for nn_DiscreteDecisionTransformer.

Decision-transformer forward: embed(a,r,s) -> LN -> +posenc, then 4 blocks of
[causal self-attn, cross-attn, FFN] with post-LN, then action head.

Distribution: data-parallel over batch, 16 batches / 8 cores = 2 per core.
Params replicated; zero collectives. Inside each core everything is
feature-major ([dmodel on partitions, tokens on free dim]) so GEMMs contract
over partitions with no transposes.

Key simplifications baked into the host prep:
 - Cross-attention has a single key/value (one task token), so softmax==1 and
   the whole cross-attn block collapses to a per-(block,batch) bias vector,
   precomputed on host and fused into LN1's beta.
 - Q-side 1/sqrt(dh) folded into Wq/bq.
 - Causal mask is additive (-30000 on the 4 diagonal-band tiles); fully
   masked key tiles are skipped outright.
 - Softmax denominators come free from the PV matmul via a ones-column
   appended to V (97-column heads); no max-subtraction needed (scores are
   O(few) by construction, exp never overflows).
 - LayerNorm stats (sum, sum-of-squares) are cross-partition reductions done
   on the PE with a ones-vector lhsT; per-token scale A=rstd and shift
   B=mu*rstd are broadcast across partitions on GpSimd.

GEMMs run in bf16 with f32 PSUM accumulation (fp32 matmul is 4x slower and
float32r locks up the device); measured end-to-end error vs the f32 reference
is <1e-2 scale-relative.
"""

import sys
from contextlib import ExitStack

sys.path.insert(0, "/opt/trn_rl_repo")

import numpy as np
import ml_dtypes

import concourse.bacc as bacc
import concourse.mybir as mybir
import concourse.tile as tile
from concourse.bass_utils import run_bass_kernel_spmd

bf = ml_dtypes.bfloat16

B, L, D, H, DH, NB, E = 16, 1024, 768, 8, 96, 4, 256
A_DIM, S_DIM = 64, 128
NCORES = 8
CPC = B // NCORES  # batches per core
KT = D // 128      # 6 k-tiles of dmodel
MT = D // 128      # 6 m-tiles of dmodel
CH = 512           # token chunk (matmul N)
NCH = L // CH      # 2 chunks per batch
FFT = 4 * D // 128 # 24 m-tiles of ffn hidden
F32, BF = mybir.dt.float32, mybir.dt.bfloat16
AL = mybir.AluOpType
AF = mybir.ActivationFunctionType

_CACHE = {}


def _rearr_pk(ap, p):
    return ap.rearrange("(k p) -> p k", p=p)


def _build():
    """Emit the full per-core program. Returns the finished Bacc object."""
    nc = bacc.Bacc("TRN2", target_bir_lowering=False, debug=False)
    dram = nc.dram_tensor

    ars = dram("ars", [CPC, 193, L], BF, kind="ExternalInput")
    wa = dram("wa", [A_DIM, E], BF, kind="ExternalInput")
    wr = dram("wr", [1, E], BF, kind="ExternalInput")
    ws = dram("ws", [S_DIM, E], BF, kind="ExternalInput")
    bemb = dram("bemb", [D], F32, kind="ExternalInput")
    lnp0 = dram("lnp0", [3, D], F32, kind="ExternalInput")
    pos = dram("pos", [D, L], F32, kind="ExternalInput")
    wq = dram("wq", [NB, D, D], BF, kind="ExternalInput")
    wk = dram("wk", [NB, D, D], BF, kind="ExternalInput")
    wv = dram("wv", [NB, D, D], BF, kind="ExternalInput")
    wo = dram("wo", [NB, D, D], BF, kind="ExternalInput")
    w1 = dram("w1", [NB, D, 4 * D], BF, kind="ExternalInput")
    w2 = dram("w2", [NB, 4 * D, D], BF, kind="ExternalInput")
    bq = dram("bq", [NB, D], F32, kind="ExternalInput")
    bk = dram("bk", [NB, D], F32, kind="ExternalInput")
    bvb = dram("bvb", [NB, 128, 8 * 97], BF, kind="ExternalInput")
    bo = dram("bo", [NB, D], F32, kind="ExternalInput")
    b1 = dram("b1", [NB, 4 * D], F32, kind="ExternalInput")
    b2 = dram("b2", [NB, D], F32, kind="ExternalInput")
    cabb = dram("cabb", [NB, CPC, D], F32, kind="ExternalInput")
    ln1g = dram("ln1g", [NB, 2, D], F32, kind="ExternalInput")
    lnp = dram("lnp", [NB, 2, 3, D], F32, kind="ExternalInput")
    masks = dram("masks", [4, 128, CH], BF, kind="ExternalInput")
    fcw = dram("fcw", [D, A_DIM], BF, kind="ExternalInput")
    fcb = dram("fcb", [A_DIM], F32, kind="ExternalInput")
    y = dram("y", [CPC, A_DIM, L], F32, kind="ExternalOutput")

    with tile.TileContext(nc) as tc, ExitStack() as ctx:
            ep = ctx.enter_context
            cst = ep(tc.tile_pool(name="cst", bufs=1))
            wblk = ep(tc.tile_pool(name="wblk", bufs=1))
            wstr = ep(tc.tile_pool(name="wstr", bufs=8))
            w2str = ep(tc.tile_pool(name="w2str", bufs=8))
            xp = ep(tc.tile_pool(name="xp", bufs=1))
            qkp = ep(tc.tile_pool(name="qk", bufs=1))
            vap = ep(tc.tile_pool(name="vap", bufs=1))
            ptp = ep(tc.tile_pool(name="ptp", bufs=9))
            otp = ep(tc.tile_pool(name="otp", bufs=1))
            scr = ep(tc.tile_pool(name="scr", bufs=3))
            hp = ep(tc.tile_pool(name="hp", bufs=1))
            smv = ep(tc.tile_pool(name="smv", bufs=3))
            abp = ep(tc.tile_pool(name="abp", bufs=1))
            bias = ep(tc.tile_pool(name="bias", bufs=1))
            pmm = ep(tc.tile_pool(name="pmm", bufs=3, space="PSUM"))
            ppv = ep(tc.tile_pool(name="ppv", bufs=2, space="PSUM"))
            pst = ep(tc.tile_pool(name="pst", bufs=1, space="PSUM"))
            # ---------- global constants ----------
            ones = cst.tile([128, 1], BF)
            nc.any.memset(ones[:], 1.0)
            epst = cst.tile([1, 1], F32)
            nc.any.memset(epst[:], 1e-5)
            maskt = []
            for rt in range(4):
                m = cst.tile([128, CH], BF, tag=f"mask{rt}")
                nc.sync.dma_start(m[:], masks[rt])
                maskt.append(m)
            fct = []
            for k in range(KT):
                t = cst.tile([128, A_DIM], BF, tag=f"fcw{k}")
                nc.sync.dma_start(t[:], fcw[k * 128:(k + 1) * 128, :])
                fct.append(t)
            fcbt = cst.tile([A_DIM, 1], F32, tag="fcb")
            nc.sync.dma_start(fcbt[:], fcb[:].rearrange("(m o) -> m o", o=1))

            # residual-stream tiles, two roles that alternate per LN
            xt = [[[xp.tile([128, L], BF, tag=f"x{b}_{j}_{k}", name=f"x{b}_{j}_{k}") for k in range(KT)]
                   for j in range(2)] for b in range(CPC)]

            def ln_chunk(b, c, IN, OUT, gt, gnt, bt_, post_pos=False):
                """LayerNorm over features for one 512-token chunk.

                IN/OUT: lists of 6 [128, L] bf16 tiles (feature-major).
                gt/gnt/bt_: [128, 6] param tiles (gamma, -gamma, beta).
                post_pos: add positional-encoding chunk after the affine step.
                """
                cs = slice(c * CH, (c + 1) * CH)
                st0 = pst.tile([1, CH], F32, tag="st0")
                st1 = pst.tile([1, CH], F32, tag="st1")
                for k in range(KT):
                    nc.tensor.matmul(st0[:], ones[:], IN[k][:, cs],
                                     start=(k == 0), stop=(k == KT - 1))
                for k in range(KT):
                    xsq = scr.tile([128, CH], BF, tag="xsq", bufs=2)
                    nc.scalar.activation(xsq[:], IN[k][:, cs], AF.Square)
                    nc.tensor.matmul(st1[:], ones[:], xsq[:],
                                     start=(k == 0), stop=(k == KT - 1))
                mu = smv.tile([1, CH], F32, tag="mu", bufs=2)
                nc.vector.tensor_scalar_mul(mu[:], st0[:], 1.0 / D)
                m2 = smv.tile([1, CH], F32, tag="sm")
                nc.vector.tensor_scalar_mul(m2[:], st1[:], 1.0 / D)
                mu2 = smv.tile([1, CH], F32, tag="sm")
                nc.vector.tensor_mul(mu2[:], mu[:], mu[:])
                var = smv.tile([1, CH], F32, tag="sm")
                nc.vector.tensor_sub(var[:], m2[:], mu2[:])
                sd = smv.tile([1, CH], F32, tag="sm")
                nc.scalar.activation(sd[:], var[:], AF.Sqrt, bias=epst[:])
                ab = abp.tile([1, 2 * CH], F32, tag="ab")
                nc.vector.reciprocal(ab[:, 0:CH], sd[:])
                nc.vector.tensor_mul(ab[:, CH:2 * CH], mu[:], ab[:, 0:CH])
                abb = abp.tile([128, 2 * CH], F32, tag="abb")
                nc.gpsimd.partition_broadcast(abb[:], ab[:])
                for k in range(KT):
                    u = scr.tile([128, CH], F32, tag="scr")
                    nc.vector.scalar_tensor_tensor(
                        u[:], IN[k][:, cs], gt[:, k:k + 1], abb[:, 0:CH],
                        op0=AL.mult, op1=AL.mult)
                    w_ = scr.tile([128, CH], F32, tag="scr")
                    nc.vector.scalar_tensor_tensor(
                        w_[:], abb[:, CH:2 * CH], gnt[:, k:k + 1], u[:],
                        op0=AL.mult, op1=AL.add)
                    if post_pos:
                        t2 = scr.tile([128, CH], F32, tag="scr")
                        nc.scalar.activation(t2[:], w_[:], AF.Identity,
                                             bias=bt_[:, k:k + 1])
                        pe = scr.tile([128, CH], F32, tag="scr")
                        nc.sync.dma_start(pe[:], pos[k * 128:(k + 1) * 128, cs])
                        nc.vector.tensor_add(OUT[k][:, cs], t2[:], pe[:])
                    else:
                        nc.scalar.activation(OUT[k][:, cs], w_[:], AF.Identity,
                                             bias=bt_[:, k:k + 1])

            # ---------- embed + LN + posenc ----------
            wat = cst.tile([A_DIM, E], BF, tag="wa")
            nc.sync.dma_start(wat[:], wa[:])
            wrt = cst.tile([1, E], BF, tag="wr")
            nc.sync.dma_start(wrt[:], wr[:])
            wst = cst.tile([S_DIM, E], BF, tag="ws")
            nc.sync.dma_start(wst[:], ws[:])
            bembt = cst.tile([128, KT], F32, tag="bemb")
            nc.sync.dma_start(bembt[:], _rearr_pk(bemb[:], 128))
            p0 = []
            for j in range(3):
                t = cst.tile([128, KT], F32, tag=f"lnp0{j}")
                nc.sync.dma_start(t[:], _rearr_pk(lnp0[j], 128))
                p0.append(t)

            for b in range(CPC):
                for c in range(NCH):
                    cs = slice(c * CH, (c + 1) * CH)
                    ta = scr.tile([A_DIM, CH], BF, tag="scr")
                    nc.sync.dma_start(ta[:], ars[b, 0:A_DIM, cs])
                    tr = scr.tile([1, CH], BF, tag="scr")
                    nc.sync.dma_start(tr[:], ars[b, A_DIM:A_DIM + 1, cs])
                    ts = scr.tile([S_DIM, CH], BF, tag="scr")
                    nc.sync.dma_start(ts[:], ars[b, A_DIM + 1:193, cs])
                    for m in range(MT):
                        p = pmm.tile([128, CH], F32, tag="mm")
                        ms = slice((m % 2) * 128, (m % 2) * 128 + 128)
                        if m < 2:
                            nc.tensor.matmul(p[:], wat[:, ms], ta[:],
                                             start=True, stop=True)
                        elif m < 4:
                            nc.tensor.matmul(p[:], wrt[:, ms], tr[:],
                                             start=True, stop=True)
                        else:
                            nc.tensor.matmul(p[:], wst[:, ms], ts[:],
                                             start=True, stop=True)
                        nc.vector.tensor_scalar_add(xt[b][0][m][:, cs], p[:],
                                                    bembt[:, m:m + 1])
                    ln_chunk(b, c, xt[b][0], xt[b][1], p0[0], p0[1], p0[2],
                             post_pos=True)

            # roles: after embed, x lives in role 1
            cur = [1, 1]

            # ---------- transformer blocks ----------
            for i in range(NB):
                wqt, wkt, wvt = [], [], []
                for k in range(KT):
                    ks = slice(k * 128, (k + 1) * 128)
                    for lst, src, tag in ((wqt, wq, "wq"), (wkt, wk, "wk"),
                                          (wvt, wv, "wv")):
                        t = wblk.tile([128, D], BF, tag=f"{tag}{k}")
                        nc.sync.dma_start(t[:], src[i, ks, :])
                        lst.append(t)
                bqt = bias.tile([DH, H], F32, tag="bq")
                nc.sync.dma_start(bqt[:], _rearr_pk(bq[i], DH))
                bkt = bias.tile([DH, H], F32, tag="bk")
                nc.sync.dma_start(bkt[:], _rearr_pk(bk[i], DH))
                bvbt = bias.tile([128, 8 * 97], BF, tag="bvb")
                nc.sync.dma_start(bvbt[:], bvb[i])
                bot = bias.tile([128, MT], F32, tag="bo")
                nc.sync.dma_start(bot[:], _rearr_pk(bo[i], 128))
                b1t = bias.tile([128, FFT], F32, tag="b1")
                nc.sync.dma_start(b1t[:], _rearr_pk(b1[i], 128))
                b2t = bias.tile([128, MT], F32, tag="b2")
                nc.sync.dma_start(b2t[:], _rearr_pk(b2[i], 128))
                cabt = []
                for b in range(CPC):
                    t = bias.tile([128, KT], F32, tag=f"cab{b}")
                    nc.sync.dma_start(t[:], _rearr_pk(cabb[i, b], 128))
                    cabt.append(t)
                l1g = bias.tile([128, KT], F32, tag="l1g")
                nc.sync.dma_start(l1g[:], _rearr_pk(ln1g[i, 0], 128))
                l1n = bias.tile([128, KT], F32, tag="l1n")
                nc.sync.dma_start(l1n[:], _rearr_pk(ln1g[i, 1], 128))
                lp = {}
                for li, lname in ((0, "l2"), (1, "l3")):
                    for j, jn in ((0, "g"), (1, "n"), (2, "b")):
                        t = bias.tile([128, KT], F32, tag=f"{lname}{jn}")
                        nc.sync.dma_start(t[:], _rearr_pk(lnp[i, li, j], 128))
                        lp[f"{lname}{jn}"] = t

                for b in range(CPC):
                    X = xt[b][cur[b]]          # block input (role j)
                    R = xt[b][1 - cur[b]]      # scratch role
                    # ---- QKV projections ----
                    qt, kt_ = [], []
                    for h in range(H):
                        tq = qkp.tile([DH, L], BF, tag=f"q{h}")
                        tk = qkp.tile([DH, L], BF, tag=f"k{h}")
                        qt.append(tq)
                        kt_.append(tk)
                    vt = []
                    for tt in range(L // 128):
                        tv = vap.tile([128, 8 * 97], BF, tag=f"v{tt}")
                        vt.append(tv)
                    for c in range(NCH):
                        cs = slice(c * CH, (c + 1) * CH)
                        for h in range(H):
                            hs = slice(h * DH, (h + 1) * DH)
                            pq = pmm.tile([DH, CH], F32, tag="mm")
                            for k in range(KT):
                                nc.tensor.matmul(pq[:], wqt[k][:, hs],
                                                 X[k][:, cs],
                                                 start=(k == 0),
                                                 stop=(k == KT - 1))
                            nc.vector.tensor_scalar_add(qt[h][:, cs], pq[:],
                                                        bqt[:, h:h + 1])
                            pk = pmm.tile([DH, CH], F32, tag="mm")
                            for k in range(KT):
                                nc.tensor.matmul(pk[:], wkt[k][:, hs],
                                                 X[k][:, cs],
                                                 start=(k == 0),
                                                 stop=(k == KT - 1))
                            nc.vector.tensor_scalar_add(kt_[h][:, cs], pk[:],
                                                        bkt[:, h:h + 1])
                        for tt in range(CH // 128):
                            tg = c * (CH // 128) + tt
                            tok = slice(tg * 128, (tg + 1) * 128)
                            for hg in range(2):
                                pv = pmm.tile([128, 4 * DH], F32, tag="mm")
                                for k in range(KT):
                                    nc.tensor.matmul(
                                        pv[:], X[k][:, tok],
                                        wvt[k][:, hg * 4 * DH:(hg + 1) * 4 * DH],
                                        start=(k == 0), stop=(k == KT - 1))
                                for hh in range(4):
                                    h = hg * 4 + hh
                                    nc.vector.scalar_tensor_tensor(
                                        vt[tg][:, h * 97:h * 97 + DH],
                                        pv[:, hh * DH:(hh + 1) * DH], 1.0,
                                        bvbt[:, h * 97:h * 97 + DH],
                                        op0=AL.mult, op1=AL.add)
                            nc.vector.tensor_copy(vt[tg][:, 96:8 * 97:97],
                                                  bvbt[:, 96:8 * 97:97])
                    # ---- attention + output proj, per chunk ----
                    for c in range(NCH):
                        cs = slice(c * CH, (c + 1) * CH)
                        ktc = 4 * (c + 1)
                        ot = []
                        for h in range(H):
                            pts = []
                            for kt2 in range(ktc):
                                ks2 = slice(kt2 * 128, (kt2 + 1) * 128)
                                psc = pmm.tile([128, CH], F32, tag="mm")
                                nc.tensor.matmul(psc[:], kt_[h][:, ks2],
                                                 qt[h][:, cs],
                                                 start=True, stop=True)
                                ptile = ptp.tile([128, CH], BF, tag="pt")
                                rt = kt2 - 4 * c
                                if rt >= 0:
                                    tmp = scr.tile([128, CH], F32, tag="scr")
                                    nc.vector.scalar_tensor_tensor(
                                        tmp[:], psc[:], 1.0, maskt[rt][:],
                                        op0=AL.mult, op1=AL.add)
                                    nc.scalar.activation(ptile[:], tmp[:], AF.Exp)
                                else:
                                    nc.scalar.activation(ptile[:], psc[:], AF.Exp)
                                pts.append(ptile)
                            po = ppv.tile([DH + 1, CH], F32, tag="pv")
                            for kt2 in range(ktc):
                                nc.tensor.matmul(
                                    po[:], vt[kt2][:, h * 97:h * 97 + 97],
                                    pts[kt2][:],
                                    start=(kt2 == 0), stop=(kt2 == ktc - 1))
                            dinv = smv.tile([1, CH], F32, tag="mu", bufs=2)
                            nc.vector.reciprocal(dinv[:], po[DH:DH + 1, :])
                            dib = abp.tile([DH, CH], F32, tag="abb")
                            nc.gpsimd.partition_broadcast(dib[:], dinv[:])
                            oht = otp.tile([DH, CH], BF, tag=f"o{h}")
                            nc.vector.scalar_tensor_tensor(
                                oht[:], po[0:DH, :], 1.0, dib[:],
                                op0=AL.mult, op1=AL.mult)
                            ot.append(oht)
                        for m in range(MT):
                            ms = slice(m * 128, (m + 1) * 128)
                            pp = pmm.tile([128, CH], F32, tag="mm")
                            for h in range(H):
                                twh = wstr.tile([DH, 128], BF, tag="wo", bufs=10)
                                nc.sync.dma_start(
                                    twh[:], wo[i, h * DH:(h + 1) * DH, ms])
                                nc.tensor.matmul(pp[:], twh[:], ot[h][:],
                                                 start=(h == 0),
                                                 stop=(h == H - 1))
                            nc.vector.scalar_tensor_tensor(
                                R[m][:, cs], pp[:], bot[:, m:m + 1],
                                X[m][:, cs], op0=AL.add, op1=AL.add)
                        # LN1 (beta fused with cross-attn bias) -> X role
                        ln_chunk(b, c, R, X, l1g, l1n, cabt[b])
                        # LN2 -> R role
                        ln_chunk(b, c, X, R, lp["l2g"], lp["l2n"], lp["l2b"])
                        # ---- FFN on R -> X role ----
                        ht = [hp.tile([128, CH], BF, tag=f"h{m}", name=f"h{m}")
                              for m in range(FFT)]
                        # stream W1 as [128,512] column blocks per k-tile
                        for mg in range(FFT // 4):
                            colg = slice(mg * CH, (mg + 1) * CH)
                            w1g = []
                            for k in range(KT):
                                t = wstr.tile([128, CH], BF, tag="w1")
                                nc.sync.dma_start(t[:], w1[i, k * 128:(k + 1) * 128, colg])
                                w1g.append(t)
                            for mi in range(4):
                                m = mg * 4 + mi
                                p1 = pmm.tile([128, CH], F32, tag="mm")
                                for k in range(KT):
                                    nc.tensor.matmul(
                                        p1[:], w1g[k][:, mi * 128:(mi + 1) * 128],
                                        R[k][:, cs],
                                        start=(k == 0), stop=(k == KT - 1))
                                nc.scalar.activation(ht[m][:], p1[:], AF.Relu,
                                                     bias=b1t[:, m:m + 1])
                        for m in range(MT):
                            ms = slice(m * 128, (m + 1) * 128)
                            p2 = pmm.tile([128, CH], F32, tag="mm")
                            for k in range(FFT):
                                t = w2str.tile([128, 128], BF, tag="w2")
                                nc.sync.dma_start(t[:], w2[i, k * 128:(k + 1) * 128, ms])
                                nc.tensor.matmul(p2[:], t[:], ht[k][:],
                                                 start=(k == 0),
                                                 stop=(k == FFT - 1))
                            nc.vector.scalar_tensor_tensor(
                                X[m][:, cs], p2[:], b2t[:, m:m + 1],
                                R[m][:, cs], op0=AL.add, op1=AL.add)
                        # LN3 -> R role
                        ln_chunk(b, c, X, R, lp["l3g"], lp["l3n"], lp["l3b"])
                    cur[b] = 1 - cur[b]

            # ---------- action head ----------
            for b in range(CPC):
                X = xt[b][cur[b]]
                for c in range(NCH):
                    cs = slice(c * CH, (c + 1) * CH)
                    pf = pmm.tile([A_DIM, CH], F32, tag="mm")
                    for k in range(KT):
                        nc.tensor.matmul(pf[:], fct[k][:], X[k][:, cs],
                                         start=(k == 0), stop=(k == KT - 1))
                    yt = scr.tile([A_DIM, CH], F32, tag="scr")
                    nc.vector.tensor_scalar_add(yt[:], pf[:], fcbt[:])
                    nc.sync.dma_start(y[b, :, cs], yt[:])

    nc.compile()
    return nc


def _posenc(length, d):
    pos_ = np.arange(length, dtype=np.float32)[:, None]
    i = np.arange(0, d, 2, dtype=np.float32)[None, :]
    ang = pos_ / np.power(np.float32(10000.0), i / np.float32(d))
    pe = np.zeros((length, d), np.float32)
    pe[:, 0::2] = np.sin(ang)
    pe[:, 1::2] = np.cos(ang)
    return pe


def _host_prep(inp):
    f32 = np.float32
    a, r, s, t = (np.asarray(inp[k]) for k in ("a", "r", "s", "t"))
    ars = np.concatenate(
        [np.asarray(a, f32), np.asarray(r, f32), np.asarray(s, f32)],
        axis=-1).transpose(0, 2, 1)  # [B, 193, L]
    ars = np.ascontiguousarray(ars).astype(bf)

    scale = f32(1.0 / np.sqrt(DH))
    sa_Wqkv = np.asarray(inp["sa_Wqkv"], f32)
    sa_bqkv = np.asarray(inp["sa_bqkv"], f32)
    wq = (sa_Wqkv[:, 0] * scale).astype(bf)
    wk = sa_Wqkv[:, 1].astype(bf)
    wv = sa_Wqkv[:, 2].astype(bf)
    bq = sa_bqkv[:, 0] * scale
    bk = sa_bqkv[:, 1]
    bv = sa_bqkv[:, 2]
    bvb = np.zeros((NB, 128, 8 * 97), f32)
    for h in range(H):
        bvb[:, :, h * 97:h * 97 + DH] = bv[:, None, h * DH:(h + 1) * DH]
        bvb[:, :, h * 97 + DH] = 1.0
    masks = np.zeros((4, 128, CH), f32)
    pcol = np.arange(128)[:, None]
    jcol = np.arange(CH)[None, :]
    for rt in range(4):
        masks[rt] = np.where(pcol + rt * 128 > jcol, f32(-30000.0), f32(0.0))

    task_table = np.asarray(inp["task_table"], f32)
    ca_Wqkv = np.asarray(inp["ca_Wqkv"], f32)
    ca_bqkv = np.asarray(inp["ca_bqkv"], f32)
    ca_Wo = np.asarray(inp["ca_Wo"], f32)
    ca_bo = np.asarray(inp["ca_bo"], f32)
    ln1_b = np.asarray(inp["ln1_b"], f32)
    enc = task_table[np.asarray(t)[:, 0]]  # [B, D]
    cab = np.zeros((NB, B, D), f32)
    for i in range(NB):
        v_ = enc @ ca_Wqkv[i, 2] + ca_bqkv[i, 2]
        cab[i] = v_ @ ca_Wo[i] + ca_bo[i]
    cabb_all = cab + ln1_b[:, None, :]  # [NB, B, D]

    ln1_g = np.asarray(inp["ln1_g"], f32)
    ln1gs = np.stack([ln1_g, -ln1_g], axis=1)  # [NB, 2, D]
    lnp_arr = np.stack([
        np.stack([np.asarray(inp["ln2_g"], f32), -np.asarray(inp["ln2_g"], f32),
                  np.asarray(inp["ln2_b"], f32)], axis=1),
        np.stack([np.asarray(inp["ln3_g"], f32), -np.asarray(inp["ln3_g"], f32),
                  np.asarray(inp["ln3_b"], f32)], axis=1),
    ], axis=1)  # [NB, 2, 3, D]
    ln_g = np.asarray(inp["ln_g"], f32)
    lnp0_arr = np.stack([ln_g, -ln_g, np.asarray(inp["ln_b"], f32)])

    shared = dict(
        wa=np.asarray(inp["Wa"], f32).astype(bf),
        wr=np.asarray(inp["Wr"], f32).astype(bf),
        ws=np.asarray(inp["Ws"], f32).astype(bf),
        bemb=np.concatenate([np.asarray(inp["ba"], f32),
                             np.asarray(inp["br"], f32),
                             np.asarray(inp["bs"], f32)]),
        lnp0=lnp0_arr,
        pos=np.ascontiguousarray(_posenc(L, D).T),
        wq=wq, wk=wk, wv=wv,
        wo=np.asarray(inp["sa_Wo"], f32).astype(bf),
        w1=np.asarray(inp["ff_W1"], f32).astype(bf),
        w2=np.asarray(inp["ff_W2"], f32).astype(bf),
        bq=bq, bk=bk, bvb=bvb.astype(bf),
        bo=np.asarray(inp["sa_bo"], f32),
        b1=np.asarray(inp["ff_b1"], f32),
        b2=np.asarray(inp["ff_b2"], f32),
        ln1g=ln1gs, lnp=lnp_arr,
        masks=masks.astype(bf),
        fcw=np.asarray(inp["fc_W"], f32).astype(bf),
        fcb=np.asarray(inp["fc_b"], f32),
    )
    in_maps = []
    for core in range(NCORES):
        m = dict(shared)
        m["ars"] = ars[core * CPC:(core + 1) * CPC]
        m["cabb"] = np.ascontiguousarray(
            cabb_all[:, core * CPC:(core + 1) * CPC])
        in_maps.append(m)
    return in_maps


def _get_nc():
    if "nc" not in _CACHE:
        _CACHE["nc"] = _build()
    return _CACHE["nc"]


def kernel(**inputs):
    nc = _get_nc()
    in_maps = _host_prep(inputs)
    res = run_bass_kernel_spmd(nc, in_maps, core_ids=list(range(NCORES)))
    out = np.zeros((B, L, A_DIM), np.float32)
    for core in range(NCORES):
        yc = res.results[core]["y"]  # [CPC, 64, L]
        for b in range(CPC):
            out[core * CPC + b] = yc[b].T
    return out
